# revision 1
# baseline (speedup 1.0000x reference)
"""Trainium2 Bass kernel for nn_AttnMatching.

Reference computes:
    emb = emb_table[1:L+1]                      # [L, D]
    attn = einsum('ld,ntd->nlt', emb, self_attn)
    out  = einsum('nlt,t->nl', attn, value_w[0])

Reassociated (identical math, fp32):
    ctx[n, d] = sum_t value_w[t] * self_attn[n, t, d]    # [N, D]  (tiny)
    out[n, l] = sum_d ctx[n, d] * emb[l, d]              # [N, L]

Memory-bound: dominant traffic is streaming the 25.6 MB embedding table.
Sharding: vocab axis L split across 8 cores (6250 cols each),
self_attn/value_w replicated, no communication. Host-side marshalling
puts each tensor in its DMA-friendly layout:
  - emb shard pre-transposed to [D=128, Lsh] (contraction dim on
    partitions; large per-partition descriptors per chunk).
  - self_attn re-laid-out d-major as attn_dT[d, n*T+t] with value_w
    broadcast to [D, T] prepended -> one [128, 1700] region with
    contiguous 6.8 KB per-partition rows.

Per-core program (default raw bacc implementation, hand-rolled sems;
a TileContext variant is kept behind K_IMPL=tile):
  - attn+w bursts first on the sync HWDGE ring (ring FIFO gives it a
    solo full-rate window); sync then streams half the emb chunks,
    gpsimd (SWDGE) streams the rest once attnw has landed.
  - ctxT[d, n] built on the DVE: one fused multiply + free-dim-reduce
    (scalar_tensor_tensor accum_out) per batch row, pipelined behind
    the attnw sub-DMAs.
  - PE: dependency-free bf16 warmup matmuls hold the HAM at 2.4 GHz,
    then fp32 mains: lhsT=ctxT [D,16] stationary, rhs = emb chunks
    [D,<=512] -> PSUM [16,<=512] -> DVE copy -> chunked store DMA on
    the scalar ring.
  - Epilogue: sem-only all-engine barrier + semaphore range clear so
    the NEFF is safe to re-execute.
"""

import os

import numpy as np

L = 50000
D = 128
T = 100
N = 16
NCORES = 8
LSH = L // NCORES          # 6250 columns per core

# knobs (env-overridable for A/B profiling)
DMA_CHUNK = int(os.environ.get("K_DMA_CHUNK", "1024"))  # emb load granularity
MM_CHUNK = 512             # matmul moving-operand / PSUM bank limit
MM_DT = os.environ.get("K_MM_DT", "float32")  # matmul input dtype mode
NUM_DEVICES = int(os.environ.get("K_NUM_DEVICES", str(NCORES)))
N_WARMUP = int(os.environ.get("K_N_WARMUP", "8"))  # PE HAM warmup matmuls
IMPL = os.environ.get("K_IMPL", "raw")  # "tile" | "raw"

_cache = {}


def _chunks(total, step):
    return [(c0, min(c0 + step, total)) for c0 in range(0, total, step)]


def _build():
    import concourse.bacc as bacc
    import concourse.mybir as mybir
    import concourse.tile as tile

    mm_dt = getattr(mybir.dt, MM_DT)

    nc = bacc.Bacc(
        "TRN2",
        target_bir_lowering=False,
        debug=False,
        enable_asserts=True,
        num_devices=NUM_DEVICES,
    )

    embT = nc.dram_tensor("embT", [D, LSH], mm_dt, kind="ExternalInput").ap()
    attnw = nc.dram_tensor(
        "attnw", [T, N * D + 1], mybir.dt.float32, kind="ExternalInput"
    ).ap()
    out = nc.dram_tensor("out", [N, LSH], mybir.dt.float32, kind="ExternalOutput").ap()

    from concourse.tile_rust import add_dep_helper

    dma_chunks = _chunks(LSH, DMA_CHUNK)
    n_sync = (len(dma_chunks) + 1) // 2

    with tile.TileContext(nc) as tc:
        with (
            tc.tile_pool(name="consts", bufs=1) as consts,
            tc.tile_pool(name="embp", bufs=len(dma_chunks)) as embp,
            tc.tile_pool(name="outp", bufs=3) as outp,
            tc.tile_pool(name="psc", bufs=1, space="PSUM") as psc,
            tc.tile_pool(name="pso", bufs=4, space="PSUM") as pso,
        ):
            # attn+w upload, layout [w | n0..n15 blocks], split into 4
            # sub-DMAs issued FIRST on the sync ring: ring FIFO gives them
            # a solo full-rate burst before the emb stream, and the ctx
            # matmuls pipeline behind the sub-DMAs via subtile deps.
            attnw_tile = consts.tile([T, N * D + 1], mybir.dt.float32)
            attnw_bounds = [0, 513, 1025, 1537, 2049]
            attnw_last = None
            for a0, a1 in zip(attnw_bounds[:-1], attnw_bounds[1:]):
                attnw_last = nc.sync.dma_start(
                    attnw_tile[:, a0:a1], attnw[:, a0:a1]
                )

            # emb chunks: first half behind attnw on the sync ring (FIFO);
            # rest on the gpsimd ring, dep-delayed behind the attnw burst
            # so round-robin doesn't starve it.
            emb_tiles = []
            for ci, (c0, c1) in enumerate(dma_chunks):
                et = embp.tile(
                    [D, c1 - c0], mm_dt, tag="emb", name=f"emb_{c0}"
                )
                eng = nc.sync if ci < n_sync else nc.gpsimd
                dma = eng.dma_start(et[:, :], embT[:, c0:c1])
                if ci == n_sync:
                    add_dep_helper(
                        attnw_last.ins, dma.ins, sync=True,
                        reason="gpsimd emb stream waits for attnw burst",
                    )
                emb_tiles.append(et)

            # PE HAM warmup: dependency-free bf16 matmuls on a zeroed
            # scratch keep the PE at 2.4 GHz until real matmuls arrive.
            if N_WARMUP:
                wscr = consts.tile([D, D + MM_CHUNK], mybir.dt.bfloat16)
                nc.vector.memset(wscr[:, :], 0.0)
                ps_w = psc.tile(
                    [D, MM_CHUNK], mybir.dt.float32, tag="ps_warm", name="ps_warm"
                )
                for wi in range(N_WARMUP):
                    nc.tensor.matmul(
                        ps_w[:, :],
                        lhsT=wscr[:, :D],
                        rhs=wscr[:, D:],
                        start=True,
                        stop=True,
                    )

            # ctxT[d, n] = sum_t self_attn[n, t, d] * w[t]
            ps_ctx = psc.tile([D, N], mybir.dt.float32)
            for n in range(N):
                nc.tensor.matmul(
                    ps_ctx[:, n : n + 1],
                    lhsT=attnw_tile[:, 1 + n * D : 1 + (n + 1) * D],
                    rhs=attnw_tile[:, 0:1],
                    start=True,
                    stop=True,
                )
            ctxT = consts.tile([D, N], mm_dt)
            nc.vector.tensor_copy(ctxT[:, :], ps_ctx[:, :])
            ctxT_mm = ctxT[:, :]

            # out[n, c0:c1] = ctxT.T @ embT[:, c0:c1]
            for ci, (c0, c1) in enumerate(dma_chunks):
                ot = outp.tile([N, c1 - c0], mybir.dt.float32, tag="out", name=f"out_{c0}")
                for s0, s1 in _chunks(c1 - c0, MM_CHUNK):
                    ps = pso.tile(
                        [N, s1 - s0], mybir.dt.float32, tag="pso", name=f"ps_{c0}_{s0}"
                    )
                    nc.tensor.matmul(
                        ps[:, :],
                        lhsT=ctxT_mm,
                        rhs=emb_tiles[ci][:, s0:s1],
                        start=True,
                        stop=True,
                    )
                    nc.vector.tensor_copy(ot[:, s0:s1], ps[:, :])
                nc.scalar.dma_start(out[:, c0:c1], ot[:, :])

    nc.compile()
    return nc


def _build_raw():
    """Raw bacc (no TileContext): hand-rolled semaphores, same schedule as
    the Tile build but with a minimal prologue/epilogue."""
    import concourse.bacc as bacc
    import concourse.mybir as mybir

    f32 = mybir.dt.float32
    bf16 = mybir.dt.bfloat16

    nc = bacc.Bacc(
        "TRN2",
        target_bir_lowering=False,
        debug=False,
        enable_asserts=True,
        num_devices=NUM_DEVICES,
    )

    embT = nc.dram_tensor("embT", [D, LSH], f32, kind="ExternalInput").ap()
    # [D, T + N*T]: cols 0..T-1 = value_w broadcast over partitions,
    # cols T.. = self_attn in d-major layout attn_dT[d, n*T+t].
    AW = T + N * T
    attnw = nc.dram_tensor("attnw", [D, AW], f32, kind="ExternalInput").ap()
    out = nc.dram_tensor("out", [N, LSH], f32, kind="ExternalOutput").ap()

    # First emb chunk small (one matmul's worth): it queues behind the
    # attnw burst on the sync ring and gates the first main matmul, so
    # keep its transfer short.
    dma_chunks = [(0, MM_CHUNK)] + [
        (c0 + MM_CHUNK, c1 + MM_CHUNK) for c0, c1 in _chunks(LSH - MM_CHUNK, DMA_CHUNK)
    ]
    n_chunks = len(dma_chunks)
    n_sync = (n_chunks + 1) // 2
    # two attnw sub-DMAs: 3.4 KB per-partition descriptors stream faster
    # than a 4-way split's 1.7 KB ones, and ctx ops still pipeline.
    attnw_bounds = [0, T + 8 * T, AW]
    n_sub = len(attnw_bounds) - 1
    CTX_PER_SUB = N // n_sub
    # gpsimd emb stream starts once this attnw sub-DMA has landed
    GP_DELAY_SUB = int(os.environ.get("K_GP_DELAY_SUB", str(n_sub - 1)))
    # global matmul list: (chunk_idx, abs_s0, abs_s1)
    mm_list = []
    for ci, (c0, c1) in enumerate(dma_chunks):
        for s0, s1 in _chunks(c1 - c0, MM_CHUNK):
            mm_list.append((ci, c0 + s0, c0 + s1))
    NPS = 4

    attnw_sb = nc.alloc_sbuf_tensor("attnw_sb", [D, AW], f32).ap()
    emb_sb = [
        nc.alloc_sbuf_tensor(f"emb_sb{ci}", [D, c1 - c0], f32).ap()
        for ci, (c0, c1) in enumerate(dma_chunks)
    ]
    out_sb = nc.alloc_sbuf_tensor("out_sb", [N, LSH], f32).ap()
    wscr = nc.alloc_sbuf_tensor("wscr", [D, D + MM_CHUNK], bf16).ap()
    ctxT = nc.alloc_sbuf_tensor("ctxT", [D, N], f32).ap()
    ctx_scr = nc.alloc_sbuf_tensor("ctx_scr", [D, N * T], f32).ap()
    ps_warm = nc.alloc_psum_tensor("ps_warm", [D, MM_CHUNK], f32).ap()
    ps_main = [
        nc.alloc_psum_tensor(f"ps_main{j}", [N, MM_CHUNK], f32).ap()
        for j in range(NPS)
    ]

    lda = [nc.alloc_semaphore(f"lda{g}") for g in range(n_sub)]
    lde = [nc.alloc_semaphore(f"lde{ci}") for ci in range(n_chunks)]
    z = nc.alloc_semaphore("z")
    cc = nc.alloc_semaphore("cc")
    mm = nc.alloc_semaphore("mm")
    cp = nc.alloc_semaphore("cp")
    st = nc.alloc_semaphore("st")
    all_sems = lda + lde + [z, cc, mm, cp, st]

    # SP load issues + DVE warmup-scratch memset go in the entry block,
    # BEFORE nc.Block(): they run right after the boot barrier instead of
    # paying the block branch + IRAM fetch first (same pattern as
    # bass_test_utils.run_sbuf_kernel's pre-block loads).
    for g, (a0, a1) in enumerate(zip(attnw_bounds[:-1], attnw_bounds[1:])):
        nc.sync.dma_start(attnw_sb[:, a0:a1], attnw[:, a0:a1]).then_inc(lda[g], 16)
    for ci in range(n_sync):
        c0, c1 = dma_chunks[ci]
        nc.sync.dma_start(emb_sb[ci][:, :], embT[:, c0:c1]).then_inc(lde[ci], 16)
    nc.vector.memset(wscr[:, :], 0.0).then_inc(z, 1)

    with nc.Block() as block:

        @block.gpsimd
        def _(gp):
            # don't compete with the attnw burst
            gp.wait_ge(lda[GP_DELAY_SUB], 16)
            for ci in range(n_sync, n_chunks):
                c0, c1 = dma_chunks[ci]
                gp.dma_start(emb_sb[ci][:, :], embT[:, c0:c1]).then_inc(
                    lde[ci], 16
                )

        @block.vector
        def _(v):
            # ctxT[:, n] = sum_t attn_dT[:, n*T+t] * w[t] — one fused
            # multiply+freedim-reduce per n on the DVE.
            for nidx in range(N):
                if nidx % CTX_PER_SUB == 0:
                    v.wait_ge(lda[nidx // CTX_PER_SUB], 16)
                inst = nc.vector.scalar_tensor_tensor(
                    ctx_scr[:, nidx * T : (nidx + 1) * T],
                    attnw_sb[:, T + nidx * T : T + (nidx + 1) * T],
                    1.0,
                    attnw_sb[:, 0:T],
                    op0=mybir.AluOpType.bypass,
                    op1=mybir.AluOpType.mult,
                    accum_out=ctxT[:, nidx : nidx + 1],
                )
            inst.then_inc(cc, 1)
            for s, (ci, s0, s1) in enumerate(mm_list):
                v.wait_ge(mm, s + 1)
                nc.vector.tensor_copy(
                    out_sb[:, s0:s1], ps_main[s % NPS][:, : s1 - s0]
                ).then_inc(cp, 1)

        @block.tensor
        def _(t):
            t.wait_ge(z, 1)
            for _wi in range(N_WARMUP):
                nc.tensor.matmul(
                    ps_warm[:, :],
                    lhsT=wscr[:, :D],
                    rhs=wscr[:, D:],
                    start=True,
                    stop=True,
                )
            t.wait_ge(cc, 1)
            prev_ci = -1
            for s, (ci, s0, s1) in enumerate(mm_list):
                if ci != prev_ci:
                    t.wait_ge(lde[ci], 16)
                    prev_ci = ci
                if s >= NPS:
                    t.wait_ge(cp, s - NPS + 1)
                c0 = dma_chunks[ci][0]
                nc.tensor.matmul(
                    ps_main[s % NPS][:, : s1 - s0],
                    lhsT=ctxT[:, :],
                    rhs=emb_sb[ci][:, s0 - c0 : s1 - c0],
                    start=True,
                    stop=True,
                ).then_inc(mm, 1)

        @block.scalar
        def _(sc):
            copies_done = 0
            for ci, (c0, c1) in enumerate(dma_chunks):
                copies_done += len(_chunks(c1 - c0, MM_CHUNK))
                sc.wait_ge(cp, copies_done)
                sc.dma_start(out[:, c0:c1], out_sb[:, c0:c1]).then_inc(st, 16)
            # no completion wait here: the epilogue's clear_and_free
            # dma_reset drains the st-associated store queue on gpsimd
            # before the NEFF can complete, guaranteeing the writes land.

    # epilogue: quiesce engines, zero sems for re-execution safety
    nc.all_engine_barrier(sem_only=True)
    nc.clear_and_free_semaphores(all_sems)

    nc.compile()
    return nc


def _get_nc():
    if "nc" not in _cache:
        _cache["nc"] = _build_raw() if IMPL == "raw" else _build()
    return _cache["nc"]


def _make_in_maps(self_attn, emb_table, value_w):
    self_attn = np.asarray(self_attn, dtype=np.float32)
    value_w = np.asarray(value_w, dtype=np.float32)
    if IMPL == "raw":
        # [D, T + N*T]: value_w broadcast, then d-major self_attn
        attnw = np.empty((D, T + N * T), dtype=np.float32)
        attnw[:, :T] = value_w[0][None, :]
        attnw[:, T:] = self_attn.transpose(2, 0, 1).reshape(D, N * T)
    else:
        # [T, 1 + N*D]: value_w first, then transposed self_attn blocks
        attnw = np.empty((T, N * D + 1), dtype=np.float32)
        attnw[:, 0] = value_w[0]
        attnw[:, 1:] = self_attn.transpose(1, 0, 2).reshape(T, N * D)
    embT = np.asarray(emb_table, dtype=np.float32)[1 : L + 1].T  # [D, L]
    return [
        {
            "embT": np.ascontiguousarray(embT[:, k * LSH : (k + 1) * LSH]),
            "attnw": attnw,
        }
        for k in range(NCORES)
    ]


def run(self_attn, emb_table, value_w, trace=False):
    from concourse.bass_utils import run_bass_kernel_spmd

    nc = _get_nc()
    in_maps = _make_in_maps(self_attn, emb_table, value_w)
    res = run_bass_kernel_spmd(nc, in_maps, list(range(NCORES)), trace=trace)
    full = np.concatenate(
        [res.results[k]["out"] for k in range(NCORES)], axis=1
    ).astype(np.float32)
    return full, res


def kernel(self_attn, mat2, traj, emb_table, value_w):
    full, _ = run(self_attn, emb_table, value_w, trace=False)
    return full



# revision 2
# speedup vs baseline: 1.5214x; 1.5214x over previous
"""Trainium2 Bass kernel for nn_AttnMatching.

Reference computes:
    emb = emb_table[1:L+1]                      # [L, D]
    attn = einsum('ld,ntd->nlt', emb, self_attn)
    out  = einsum('nlt,t->nl', attn, value_w[0])

Reassociated (identical math):
    ctx[n, d] = sum_t value_w[t] * self_attn[n, t, d]    # [N, D]  (tiny:
              #  0.1% of total FLOPs -> folded on host during marshalling)
    out[n, l] = sum_d ctx[n, d] * emb[l, d]              # [N, L]

Memory-bound: dominant traffic is streaming the embedding table. All
device traffic is bf16 (rel_norm vs fp32 reference ~3e-3, gate is 2e-2):
the emb shard halves to 1.6 MB/core and the PE streams bf16 4x faster
than fp32.

Sharding: vocab axis L split across 8 cores (6250 cols each), no
communication. Host-side marshalling per core: one input tensor
embx = [ctxT | embT-shard] as [D=128, 16+6250] bf16 (contraction dim on
partitions), output [N, 6250] bf16 upcast and concatenated on host.

Per-core program (raw bacc, hand-rolled sems):
  - all emb-chunk loads issued in the entry block, spread round-robin
    across the sync/scalar HWDGE rings and the gpsimd SWDGE ring;
    chunk 0 is small (ctxT + one matmul's cols) so compute starts early.
  - PE: a few dependency-free bf16 warmup matmuls on zeroed scratch to
    lift the HAM clock gate, then mains: lhsT = ctxT [128,16] from the
    chunk-0 region, rhs = emb cols [128,<=512] -> PSUM [16,<=512],
    8-bank rotation.
  - PSUM -> SBUF bf16 copies alternate between DVE (even) and ACT (odd)
    so neither engine is the copy bottleneck.
  - stores: chunked [16, cols] bf16 DMAs on the sync ring (its loads are
    long done), gated on the copy sems.
  - Epilogue: sem-only all-engine barrier + semaphore clear so the NEFF
    is safe to re-execute.
"""

import os

import numpy as np
import ml_dtypes

L = 50000
D = 128
T = 100
N = 16
NCORES = 8
LSH = L // NCORES          # 6250 columns per core
CTX = 16                   # ctxT [D, N] prepended to the emb shard
MM = 512                   # matmul moving-operand / PSUM bank limit

# knobs (env-overridable for A/B profiling)
DMA_CHUNK = int(os.environ.get("K_DMA_CHUNK", "1024"))  # emb load granularity
N_WARMUP = int(os.environ.get("K_N_WARMUP", "4"))       # PE HAM warmup matmuls
NST = int(os.environ.get("K_NST", "4"))                 # output store DMAs
RINGS = os.environ.get("K_RINGS", "sag")  # chunk ring cycle: s/a/g
NPS = int(os.environ.get("K_NPS", "8"))                 # PSUM banks in rotation
NUM_DEVICES = int(os.environ.get("K_NUM_DEVICES", str(NCORES)))

_cache = {}


def _chunks(total, step, start=0):
    return [(c0, min(c0 + step, total)) for c0 in range(start, total, step)]


def _build():
    import concourse.bacc as bacc
    import concourse.mybir as mybir

    f32 = mybir.dt.float32
    bf16 = mybir.dt.bfloat16

    nc = bacc.Bacc(
        "TRN2",
        target_bir_lowering=False,
        debug=False,
        enable_asserts=True,
        num_devices=NUM_DEVICES,
    )

    AW = CTX + LSH
    embx = nc.dram_tensor("embx", [D, AW], bf16, kind="ExternalInput").ap()
    out = nc.dram_tensor("out", [N, LSH], bf16, kind="ExternalOutput").ap()

    # DMA chunks over embx cols; chunk 0 = ctxT + first matmul's cols.
    ch = [(0, CTX + MM)] + _chunks(AW, DMA_CHUNK, start=CTX + MM)
    nch = len(ch)
    ring_names = {"s": "sync", "a": "scalar", "g": "gpsimd"}
    rings = ["sync"] + [
        ring_names[RINGS[(i - 1) % len(RINGS)]] for i in range(1, nch)
    ]

    # matmul s covers out cols [s*MM, min((s+1)*MM, LSH))
    mm_cols = _chunks(LSH, MM)
    n_mm = len(mm_cols)

    def gate(c1):
        """index of the chunk whose completion makes embx col CTX+c1 valid"""
        for i, (a, b) in enumerate(ch):
            if b >= CTX + c1:
                return i
        raise AssertionError(c1)

    gates = [gate(c1) for (_c0, c1) in mm_cols]

    embx_sb = nc.alloc_sbuf_tensor("embx_sb", [D, AW], bf16).ap()
    out_sb = nc.alloc_sbuf_tensor("out_sb", [N, LSH], bf16).ap()
    wscr = nc.alloc_sbuf_tensor("wscr", [D, CTX + MM], bf16).ap()
    ps = [
        nc.alloc_psum_tensor(f"ps{j}", [N, MM], f32).ap() for j in range(NPS)
    ]

    lde = [nc.alloc_semaphore(f"lde{i}") for i in range(nch)]
    z = nc.alloc_semaphore("z")
    mm_sem = nc.alloc_semaphore("mm")
    cpv = nc.alloc_semaphore("cpv")
    cpa = nc.alloc_semaphore("cpa")
    st = nc.alloc_semaphore("st")
    all_sems = lde + [z, mm_sem, cpv, cpa, st]

    eng = {"sync": nc.sync, "scalar": nc.scalar, "gpsimd": nc.gpsimd}

    # entry block: all loads + warmup-scratch memset run right after the
    # boot barrier, before the block branch + IRAM fetch.
    for i, (a, b) in enumerate(ch):
        eng[rings[i]].dma_start(embx_sb[:, a:b], embx[:, a:b]).then_inc(
            lde[i], 16
        )
    nc.vector.memset(wscr[:, :], 0.0).then_inc(z, 1)

    # store groups: split the matmul list into NST contiguous runs
    bounds = [round(g * n_mm / NST) for g in range(NST + 1)]
    store_groups = [
        (bounds[g], bounds[g + 1])
        for g in range(NST)
        if bounds[g + 1] > bounds[g]
    ]

    with nc.Block() as block:

        @block.tensor
        def _(t):
            t.wait_ge(z, 1)
            for _wi in range(N_WARMUP):
                nc.tensor.matmul(
                    ps[NPS - 1][:, :],
                    lhsT=wscr[:, :CTX],
                    rhs=wscr[:, CTX:],
                    start=True,
                    stop=True,
                )
            prev_gate = -1
            for s, (c0, c1) in enumerate(mm_cols):
                if gates[s] != prev_gate:
                    t.wait_ge(lde[gates[s]], 16)
                    prev_gate = gates[s]
                if s >= NPS:
                    prev = s - NPS
                    if prev % 2 == 0:
                        t.wait_ge(cpv, prev // 2 + 1)
                    else:
                        t.wait_ge(cpa, prev // 2 + 1)
                nc.tensor.matmul(
                    ps[s % NPS][:, : c1 - c0],
                    lhsT=embx_sb[:, :CTX],
                    rhs=embx_sb[:, CTX + c0 : CTX + c1],
                    start=True,
                    stop=True,
                ).then_inc(mm_sem, 1)

        @block.vector
        def _(v):
            for s in range(0, n_mm, 2):
                v.wait_ge(mm_sem, s + 1)
                c0, c1 = mm_cols[s]
                nc.vector.tensor_copy(
                    out_sb[:, c0:c1], ps[s % NPS][:, : c1 - c0]
                ).then_inc(cpv, 1)

        @block.scalar
        def _(sc):
            for s in range(1, n_mm, 2):
                sc.wait_ge(mm_sem, s + 1)
                c0, c1 = mm_cols[s]
                nc.scalar.copy(
                    out_sb[:, c0:c1], ps[s % NPS][:, : c1 - c0]
                ).then_inc(cpa, 1)

        @block.sync
        def _(sy):
            for m0, m1 in store_groups:
                n_even = (m1 + 1) // 2  # even-index mms < m1 (DVE copies)
                n_odd = m1 // 2         # odd-index mms  < m1 (ACT copies)
                if n_even:
                    sy.wait_ge(cpv, n_even)
                if n_odd:
                    sy.wait_ge(cpa, n_odd)
                a = mm_cols[m0][0]
                b = mm_cols[m1 - 1][1]
                sy.dma_start(out[:, a:b], out_sb[:, a:b]).then_inc(st, 16)
            sy.wait_ge(st, 16 * len(store_groups))

    # epilogue: quiesce engines, zero sems for re-execution safety
    nc.all_engine_barrier(sem_only=True)
    nc.clear_and_free_semaphores(all_sems)

    nc.compile()
    return nc


def _get_nc():
    if "nc" not in _cache:
        _cache["nc"] = _build()
    return _cache["nc"]


def _make_in_maps(self_attn, emb_table, value_w):
    bf = ml_dtypes.bfloat16
    sa = np.asarray(self_attn, dtype=np.float32)
    w = np.asarray(value_w, dtype=np.float32)[0]
    ctxT = np.einsum("ntd,t->dn", sa, w).astype(bf)          # [D, N]
    embT = np.asarray(emb_table, dtype=np.float32)[1 : L + 1].T.astype(bf)
    maps = []
    for k in range(NCORES):
        embx = np.empty((D, CTX + LSH), dtype=bf)
        embx[:, :CTX] = ctxT
        embx[:, CTX:] = embT[:, k * LSH : (k + 1) * LSH]
        maps.append({"embx": embx})
    return maps


def run(self_attn, emb_table, value_w, trace=False):
    from concourse.bass_utils import run_bass_kernel_spmd

    nc = _get_nc()
    in_maps = _make_in_maps(self_attn, emb_table, value_w)
    res = run_bass_kernel_spmd(nc, in_maps, list(range(NCORES)), trace=trace)
    full = np.concatenate(
        [np.asarray(res.results[k]["out"]) for k in range(NCORES)], axis=1
    ).astype(np.float32)
    return full, res


def kernel(self_attn, mat2, traj, emb_table, value_w):
    full, _ = run(self_attn, emb_table, value_w, trace=False)
    return full


# revision 3
# speedup vs baseline: 1.5830x; 1.0405x over previous
"""Trainium2 Bass kernel for nn_AttnMatching.

Reference computes:
    emb = emb_table[1:L+1]                      # [L, D]
    attn = einsum('ld,ntd->nlt', emb, self_attn)
    out  = einsum('nlt,t->nl', attn, value_w[0])

Reassociated (identical math):
    ctx[n, d] = sum_t value_w[t] * self_attn[n, t, d]    # [N, D]  (tiny:
              #  0.1% of total FLOPs -> folded on host during marshalling)
    out[n, l] = sum_d ctx[n, d] * emb[l, d]              # [N, L]

Memory-bound: dominant traffic is streaming the embedding table. All
device traffic is bf16 (rel_norm vs fp32 reference ~3e-3, gate is 2e-2):
the emb shard halves to 1.6 MB/core and the PE streams bf16 ~3x faster
than fp32.

Sharding: vocab axis L split across 8 cores (6250 cols each), no
communication. Host-side marshalling per core: the ctxT [D,16] block and
each DMA chunk of the emb shard are separate contiguous DRAM tensors
(sequential HBM reads, 4 KB packets); outputs are per-store-group
contiguous DRAM tensors, concatenated and upcast on host.

Per-core program (raw bacc, hand-rolled sems):
  - chunk loads issued in the entry block, alternating sync/scalar HWDGE
    rings so chunks complete in need-order (SDMA round-robins rings at
    packet granularity; the next-needed chunk is always at the head of
    the other ring). Chunk 0 (ctxT + one matmul of cols) is small so
    compute starts early.
  - PE: dependency-free bf16 warmup matmuls on scratch (uninitialized --
    only timing matters) bridge the HAM clock-gate window until data
    lands, then mains: lhsT = ctxT [128,16], rhs = emb cols [128,<=512]
    -> PSUM [16,<=512], 8-bank rotation.
  - PSUM -> SBUF bf16 copies alternate between DVE (even) and ACT (odd);
    the two engines touch disjoint PSUM banks.
  - stores: per-group [16, cols] bf16 DMAs on the gpsimd SWDGE ring
    (otherwise idle), gated on the copy sems. No completion wait: the
    epilogue's clear_and_free dma_reset drains the store queue before
    the NEFF can complete.
  - Epilogue: sem-only all-engine barrier + semaphore clear so the NEFF
    is safe to re-execute.
"""

import os

import numpy as np
import ml_dtypes

L = 50000
D = 128
T = 100
N = 16
NCORES = 8
LSH = L // NCORES          # 6250 columns per core
CTX = 16                   # ctxT [D, N] prepended to chunk 0
MM = 512                   # matmul moving-operand / PSUM bank limit

# knobs (env-overridable for A/B profiling)
DMA_CHUNK = int(os.environ.get("K_DMA_CHUNK", "2048"))  # emb load cols/chunk
N_WARMUP = int(os.environ.get("K_N_WARMUP", "6"))       # PE HAM warmup matmuls
NST = int(os.environ.get("K_NST", "4"))                 # output store DMAs
NPS = int(os.environ.get("K_NPS", "8"))                 # PSUM banks in rotation
LOAD_RINGS = os.environ.get("K_LOAD_RINGS", "sa")       # ring cycle for chunks
STORE_RING = os.environ.get("K_STORE_RING", "g")        # ring for stores
NUM_DEVICES = int(os.environ.get("K_NUM_DEVICES", str(NCORES)))

_cache = {}


def _chunks(total, step, start=0):
    return [(c0, min(c0 + step, total)) for c0 in range(start, total, step)]


def _plan():
    # chunk i covers emb cols [a, b); chunk 0 also carries ctxT
    ch = [(0, MM)] + _chunks(LSH, DMA_CHUNK, start=MM)
    mm_cols = _chunks(LSH, MM)
    gates = []
    for _c0, c1 in mm_cols:
        gates.append(next(i for i, (_a, b) in enumerate(ch) if b >= c1))
    n_mm = len(mm_cols)
    bounds = [round(g * n_mm / NST) for g in range(NST + 1)]
    store_groups = [
        (bounds[g], bounds[g + 1])
        for g in range(NST)
        if bounds[g + 1] > bounds[g]
    ]
    return ch, mm_cols, gates, store_groups


def _build():
    import concourse.bacc as bacc
    import concourse.mybir as mybir

    f32 = mybir.dt.float32
    bf16 = mybir.dt.bfloat16

    nc = bacc.Bacc(
        "TRN2",
        target_bir_lowering=False,
        debug=False,
        enable_asserts=True,
        num_devices=NUM_DEVICES,
    )

    ch, mm_cols, gates, store_groups = _plan()
    nch = len(ch)
    n_mm = len(mm_cols)

    # one contiguous DRAM tensor per load chunk / store group
    emb_t = []
    for i, (a, b) in enumerate(ch):
        cols = (CTX if i == 0 else 0) + (b - a)
        emb_t.append(
            nc.dram_tensor(f"emb{i}", [D, cols], bf16, kind="ExternalInput").ap()
        )
    out_t = []
    for g, (m0, m1) in enumerate(store_groups):
        cols = mm_cols[m1 - 1][1] - mm_cols[m0][0]
        out_t.append(
            nc.dram_tensor(f"out{g}", [N, cols], bf16, kind="ExternalOutput").ap()
        )

    embx_sb = nc.alloc_sbuf_tensor("embx_sb", [D, CTX + LSH], bf16).ap()
    out_sb = nc.alloc_sbuf_tensor("out_sb", [N, LSH], bf16).ap()
    wscr = nc.alloc_sbuf_tensor("wscr", [D, CTX + MM], bf16).ap()
    ps = [
        nc.alloc_psum_tensor(f"ps{j}", [N, MM], f32).ap() for j in range(NPS)
    ]

    lde = [nc.alloc_semaphore(f"lde{i}") for i in range(nch)]
    mm_sem = nc.alloc_semaphore("mm")
    cpv = nc.alloc_semaphore("cpv")
    cpa = nc.alloc_semaphore("cpa")
    st = nc.alloc_semaphore("st")
    all_sems = lde + [mm_sem, cpv, cpa, st]

    eng = {"s": nc.sync, "a": nc.scalar, "g": nc.gpsimd}

    # entry block: all chunk loads, alternating rings, need-order heads
    for i, (a, b) in enumerate(ch):
        ring = eng[LOAD_RINGS[i % len(LOAD_RINGS)]]
        s0 = 0 if i == 0 else CTX + a
        s1 = CTX + b
        ring.dma_start(embx_sb[:, s0:s1], emb_t[i][:, :]).then_inc(lde[i], 16)

    with nc.Block() as block:

        @block.tensor
        def _(t):
            for _wi in range(N_WARMUP):
                nc.tensor.matmul(
                    ps[NPS - 1][:, :],
                    lhsT=wscr[:, :CTX],
                    rhs=wscr[:, CTX:],
                    start=True,
                    stop=True,
                )
            prev_gate = -1
            for s, (c0, c1) in enumerate(mm_cols):
                if gates[s] != prev_gate:
                    t.wait_ge(lde[gates[s]], 16)
                    prev_gate = gates[s]
                if s >= NPS:
                    prev = s - NPS
                    if prev % 2 == 0:
                        t.wait_ge(cpv, prev // 2 + 1)
                    else:
                        t.wait_ge(cpa, prev // 2 + 1)
                nc.tensor.matmul(
                    ps[s % NPS][:, : c1 - c0],
                    lhsT=embx_sb[:, :CTX],
                    rhs=embx_sb[:, CTX + c0 : CTX + c1],
                    start=True,
                    stop=True,
                ).then_inc(mm_sem, 1)

        @block.vector
        def _(v):
            for s in range(0, n_mm, 2):
                v.wait_ge(mm_sem, s + 1)
                c0, c1 = mm_cols[s]
                nc.vector.tensor_copy(
                    out_sb[:, c0:c1], ps[s % NPS][:, : c1 - c0]
                ).then_inc(cpv, 1)

        @block.scalar
        def _(sc):
            for s in range(1, n_mm, 2):
                sc.wait_ge(mm_sem, s + 1)
                c0, c1 = mm_cols[s]
                nc.scalar.copy(
                    out_sb[:, c0:c1], ps[s % NPS][:, : c1 - c0]
                ).then_inc(cpa, 1)

        @block.gpsimd
        def _(gp):
            ring = eng[STORE_RING]
            for g, (m0, m1) in enumerate(store_groups):
                n_even = (m1 + 1) // 2  # even-index mms < m1 (DVE copies)
                n_odd = m1 // 2         # odd-index mms  < m1 (ACT copies)
                if n_even:
                    gp.wait_ge(cpv, n_even)
                if n_odd:
                    gp.wait_ge(cpa, n_odd)
                a = mm_cols[m0][0]
                b = mm_cols[m1 - 1][1]
                ring.dma_start(out_t[g][:, :], out_sb[:, a:b]).then_inc(st, 16)

    # epilogue: quiesce engines, zero sems for re-execution safety; the
    # clear's dma_reset drains the store queue so the final writes land.
    nc.all_engine_barrier(sem_only=True)
    nc.clear_and_free_semaphores(all_sems)

    nc.compile()
    return nc


def _get_nc():
    if "nc" not in _cache:
        _cache["nc"] = _build()
    return _cache["nc"]


def _make_in_maps(self_attn, emb_table, value_w):
    bf = ml_dtypes.bfloat16
    sa = np.asarray(self_attn, dtype=np.float32)
    w = np.asarray(value_w, dtype=np.float32)[0]
    ctxT = np.einsum("ntd,t->dn", sa, w).astype(bf)          # [D, N]
    embT = np.asarray(emb_table, dtype=np.float32)[1 : L + 1].T.astype(bf)
    ch, _mm_cols, _gates, _sg = _plan()
    maps = []
    for k in range(NCORES):
        shard = embT[:, k * LSH : (k + 1) * LSH]
        m = {}
        for i, (a, b) in enumerate(ch):
            if i == 0:
                blk = np.concatenate([ctxT, shard[:, a:b]], axis=1)
            else:
                blk = shard[:, a:b]
            m[f"emb{i}"] = np.ascontiguousarray(blk)
        maps.append(m)
    return maps


def run(self_attn, emb_table, value_w, trace=False):
    from concourse.bass_utils import run_bass_kernel_spmd

    nc = _get_nc()
    in_maps = _make_in_maps(self_attn, emb_table, value_w)
    res = run_bass_kernel_spmd(nc, in_maps, list(range(NCORES)), trace=trace)
    _ch, _mm, _g, store_groups = _plan()
    full = np.concatenate(
        [
            np.concatenate(
                [np.asarray(res.results[k][f"out{g}"]) for g in range(len(store_groups))],
                axis=1,
            )
            for k in range(NCORES)
        ],
        axis=1,
    ).astype(np.float32)
    return full, res


def kernel(self_attn, mat2, traj, emb_table, value_w):
    full, _ = run(self_attn, emb_table, value_w, trace=False)
    return full


# revision 6
# speedup vs baseline: 1.8557x; 1.1723x over previous
"""Trainium2 Bass kernel for nn_AttnMatching.

Reference computes:
    emb = emb_table[1:L+1]                      # [L, D]
    attn = einsum('ld,ntd->nlt', emb, self_attn)
    out  = einsum('nlt,t->nl', attn, value_w[0])

Reassociated (identical math):
    ctx[n, d] = sum_t value_w[t] * self_attn[n, t, d]    # [N, D]  (tiny:
              #  0.1% of total FLOPs -> folded on host during marshalling)
    out[n, l] = sum_d ctx[n, d] * emb[l, d]              # [N, L]

Memory-bound: dominant traffic is streaming the embedding table. All
device traffic is bf16 (rel_norm vs fp32 reference ~3e-3, gate is 2e-2):
the emb shard halves to 1.6 MB/core and the PE streams bf16 ~3x faster
than fp32.

Sharding: vocab axis L split across 8 cores (6250 cols each), no
communication. Host-side marshalling per core: the ctxT [D,16] block and
each DMA chunk of the emb shard are separate contiguous DRAM tensors
(sequential HBM reads, 4 KB packets); outputs are per-store-group
contiguous DRAM tensors, concatenated and upcast on host.

Per-core program (raw bacc, hand-rolled sems):
  - chunk loads issued in the entry block, alternating sync/scalar HWDGE
    rings so chunks complete in need-order (SDMA round-robins rings at
    packet granularity; the next-needed chunk is always at the head of
    the other ring). Chunk 0 (ctxT + one matmul of cols) is small so
    compute starts early.
  - PE: dependency-free bf16 warmup matmuls on scratch (uninitialized --
    only timing matters) bridge the HAM clock-gate window until data
    lands, then mains: lhsT = ctxT [128,16], rhs = emb cols [128,<=512]
    -> PSUM [16,<=512], 8-bank rotation.
  - PSUM -> SBUF bf16 copies alternate between DVE (even) and ACT (odd);
    the two engines touch disjoint PSUM banks.
  - stores: per-group [16, cols] bf16 DMAs on the gpsimd SWDGE ring
    (otherwise idle), gated on the copy sems. No completion wait: the
    epilogue's clear_and_free dma_reset drains the store queue before
    the NEFF can complete.
  - Epilogue: sem-only all-engine barrier + semaphore clear so the NEFF
    is safe to re-execute.
"""

import os

import numpy as np
import ml_dtypes

L = 50000
D = 128
T = 100
N = 16
NCORES = 8
LSH = L // NCORES          # 6250 columns per core
CTX = 16                   # ctxT [D, N] prepended to chunk 0
MM = 512                   # matmul moving-operand / PSUM bank limit

# knobs (env-overridable for A/B profiling)
DMA_CHUNK = int(os.environ.get("K_DMA_CHUNK", "1024"))  # emb load cols/chunk
N_WARMUP = int(os.environ.get("K_N_WARMUP", "5"))       # PE HAM warmup matmuls
NPS = int(os.environ.get("K_NPS", "8"))                 # PSUM banks in rotation
LOAD_RINGS = os.environ.get("K_LOAD_RINGS", "sag")      # ring cycle for chunks
STORE_RINGS = os.environ.get("K_STORE_RINGS", "as")     # ring cycle for stores
# store groups as mm-index boundaries; last group tiny so the final
# store (the tail) is short
STORE_BOUNDS = [
    int(x) for x in os.environ.get("K_STORE_BOUNDS", "0,5,9,12,13").split(",")
]
NUM_DEVICES = int(os.environ.get("K_NUM_DEVICES", str(NCORES)))

_cache = {}


def _chunks(total, step, start=0):
    return [(c0, min(c0 + step, total)) for c0 in range(start, total, step)]


def _plan():
    # chunk i covers emb cols [a, b); chunk 0 also carries ctxT
    ch = [(0, MM)] + _chunks(LSH, DMA_CHUNK, start=MM)
    mm_cols = _chunks(LSH, MM)
    gates = []
    for _c0, c1 in mm_cols:
        gates.append(next(i for i, (_a, b) in enumerate(ch) if b >= c1))
    n_mm = len(mm_cols)
    bounds = [min(b, n_mm) for b in STORE_BOUNDS]
    assert bounds[0] == 0 and bounds[-1] == n_mm, bounds
    store_groups = [
        (bounds[g], bounds[g + 1])
        for g in range(len(bounds) - 1)
        if bounds[g + 1] > bounds[g]
    ]
    return ch, mm_cols, gates, store_groups


def _build():
    import concourse.bacc as bacc
    import concourse.mybir as mybir

    f32 = mybir.dt.float32
    bf16 = mybir.dt.bfloat16

    nc = bacc.Bacc(
        "TRN2",
        target_bir_lowering=False,
        debug=False,
        enable_asserts=True,
        num_devices=NUM_DEVICES,
    )

    ch, mm_cols, gates, store_groups = _plan()
    nch = len(ch)
    n_mm = len(mm_cols)

    # one contiguous DRAM tensor per load chunk / store group
    emb_t = []
    for i, (a, b) in enumerate(ch):
        cols = (CTX if i == 0 else 0) + (b - a)
        emb_t.append(
            nc.dram_tensor(f"emb{i}", [D, cols], bf16, kind="ExternalInput").ap()
        )
    out_t = []
    for g, (m0, m1) in enumerate(store_groups):
        cols = mm_cols[m1 - 1][1] - mm_cols[m0][0]
        out_t.append(
            nc.dram_tensor(f"out{g}", [N, cols], bf16, kind="ExternalOutput").ap()
        )

    embx_sb = nc.alloc_sbuf_tensor("embx_sb", [D, CTX + LSH], bf16).ap()
    out_sb = nc.alloc_sbuf_tensor("out_sb", [N, LSH], bf16).ap()
    wscr = nc.alloc_sbuf_tensor("wscr", [D, CTX + MM], bf16).ap()
    ps = [
        nc.alloc_psum_tensor(f"ps{j}", [N, MM], f32).ap() for j in range(NPS)
    ]

    lde = [nc.alloc_semaphore(f"lde{i}") for i in range(nch)]
    mm_sem = nc.alloc_semaphore("mm")
    cpv = nc.alloc_semaphore("cpv")
    cpa = nc.alloc_semaphore("cpa")
    st = nc.alloc_semaphore("st")
    all_sems = lde + [mm_sem, cpv, cpa, st]

    eng = {"s": nc.sync, "a": nc.scalar, "g": nc.gpsimd}

    # entry block: all chunk loads, alternating rings, need-order heads
    for i, (a, b) in enumerate(ch):
        ring = eng[LOAD_RINGS[i % len(LOAD_RINGS)]]
        s0 = 0 if i == 0 else CTX + a
        s1 = CTX + b
        ring.dma_start(embx_sb[:, s0:s1], emb_t[i][:, :]).then_inc(lde[i], 16)

    with nc.Block() as block:

        @block.tensor
        def _(t):
            for _wi in range(N_WARMUP):
                nc.tensor.matmul(
                    ps[NPS - 1][:, :],
                    lhsT=wscr[:, :CTX],
                    rhs=wscr[:, CTX:],
                    start=True,
                    stop=True,
                )
            prev_gate = -1
            for s, (c0, c1) in enumerate(mm_cols):
                if gates[s] != prev_gate:
                    t.wait_ge(lde[gates[s]], 16)
                    prev_gate = gates[s]
                if s >= NPS:
                    prev = s - NPS
                    if prev % 2 == 0:
                        t.wait_ge(cpv, prev // 2 + 1)
                    else:
                        t.wait_ge(cpa, prev // 2 + 1)
                nc.tensor.matmul(
                    ps[s % NPS][:, : c1 - c0],
                    lhsT=embx_sb[:, :CTX],
                    rhs=embx_sb[:, CTX + c0 : CTX + c1],
                    start=True,
                    stop=True,
                ).then_inc(mm_sem, 1)

        @block.vector
        def _(v):
            for s in range(0, n_mm, 2):
                v.wait_ge(mm_sem, s + 1)
                c0, c1 = mm_cols[s]
                nc.vector.tensor_copy(
                    out_sb[:, c0:c1], ps[s % NPS][:, : c1 - c0]
                ).then_inc(cpv, 1)

        # store group g needs every copy with mm-index < m1:
        #   cpv >= ceil(m1/2) (DVE, even mms), cpa >= m1//2 (ACT, odd mms)
        # groups ride the HWDGE rings per STORE_RINGS; scalar-ring stores
        # are interleaved into the ACT copy stream right after ACT's own
        # contribution completes, sync-ring stores issue from the (idle
        # after loads) sync engine.
        store_rings = [
            STORE_RINGS[g % len(STORE_RINGS)] for g in range(len(store_groups))
        ]

        def _store(issuer, g):
            m0, m1 = store_groups[g]
            n_even = (m1 + 1) // 2
            if n_even:
                issuer.wait_ge(cpv, n_even)
            n_odd = m1 // 2
            if n_odd and store_rings[g] != "a":
                issuer.wait_ge(cpa, n_odd)
            a = mm_cols[m0][0]
            b = mm_cols[m1 - 1][1]
            eng[store_rings[g]].dma_start(
                out_t[g][:, :], out_sb[:, a:b]
            ).then_inc(st, 16)

        @block.scalar
        def _(sc):
            act_stores = {}  # after ACT copy s, issue these groups
            for g, (m0, m1) in enumerate(store_groups):
                if store_rings[g] == "a":
                    n_odd = m1 // 2
                    act_stores.setdefault(
                        2 * n_odd - 1 if n_odd else -1, []
                    ).append(g)
            for g in act_stores.get(-1, []):
                _store(sc, g)
            for s in range(1, n_mm, 2):
                sc.wait_ge(mm_sem, s + 1)
                c0, c1 = mm_cols[s]
                nc.scalar.copy(
                    out_sb[:, c0:c1], ps[s % NPS][:, : c1 - c0]
                ).then_inc(cpa, 1)
                for g in act_stores.get(s, []):
                    _store(sc, g)

        @block.sync
        def _(sy):
            for g in range(len(store_groups)):
                if store_rings[g] == "s":
                    _store(sy, g)

    # epilogue: quiesce engines, zero sems for re-execution safety; the
    # clear's dma_reset drains the store queue so the final writes land.
    nc.all_engine_barrier(sem_only=True)
    nc.clear_and_free_semaphores(all_sems)

    nc.compile()
    return nc


def _get_nc():
    if "nc" not in _cache:
        _cache["nc"] = _build()
    return _cache["nc"]


def _make_in_maps(self_attn, emb_table, value_w):
    bf = ml_dtypes.bfloat16
    sa = np.asarray(self_attn, dtype=np.float32)
    w = np.asarray(value_w, dtype=np.float32)[0]
    ctxT = np.einsum("ntd,t->dn", sa, w).astype(bf)          # [D, N]
    embT = np.asarray(emb_table, dtype=np.float32)[1 : L + 1].T.astype(bf)
    ch, _mm_cols, _gates, _sg = _plan()
    maps = []
    for k in range(NCORES):
        shard = embT[:, k * LSH : (k + 1) * LSH]
        m = {}
        for i, (a, b) in enumerate(ch):
            if i == 0:
                blk = np.concatenate([ctxT, shard[:, a:b]], axis=1)
            else:
                blk = shard[:, a:b]
            m[f"emb{i}"] = np.ascontiguousarray(blk)
        maps.append(m)
    return maps


def run(self_attn, emb_table, value_w, trace=False):
    from concourse.bass_utils import run_bass_kernel_spmd

    nc = _get_nc()
    in_maps = _make_in_maps(self_attn, emb_table, value_w)
    res = run_bass_kernel_spmd(nc, in_maps, list(range(NCORES)), trace=trace)
    _ch, _mm, _g, store_groups = _plan()
    full = np.concatenate(
        [
            np.concatenate(
                [np.asarray(res.results[k][f"out{g}"]) for g in range(len(store_groups))],
                axis=1,
            )
            for k in range(NCORES)
        ],
        axis=1,
    ).astype(np.float32)
    return full, res


def kernel(self_attn, mat2, traj, emb_table, value_w):
    full, _ = run(self_attn, emb_table, value_w, trace=False)
    return full
